# revision 4
# baseline (speedup 1.0000x reference)
"""Trainium2 Bass kernel for BlockAttnRes.compute_all_inputs.

Math: for each row (b,t), layer l attends over a small per-row source stack
(embedding, completed block sums S_k, and the running partial sum). Every
source is a prefix-sum of the 25 "raw" per-row vectors X = [emb, f_0..f_23],
i.e. sources V = M @ X for a constant 0/1 matrix M (25x25). Likewise the
output h_l = sum_n alpha_{l,n} v_n = (A M) @ X, and the score dots
v_n . qw_l = M @ (X @ qw^T). So the whole layer loop collapses into a few
small matmuls per row batch - no sequential layer recurrence on device.

Device layout: batches of R=5 rows; partition p = r*25 + j (r-major), j in
[0, 25) raw index, so P = 125 partitions. Inputs are host-transposed to
[row, j, d] so each batch loads with ONE contiguous DMA; the output is
written [row, l, d] and host-transposed back. Per batch:
  1. DMA X [125, 2048] fp32 (1MB contiguous)
  2. PE transposes X chunks -> X^T (fp32), ACT copies to SBUF as bf16
  3. PE: per d-chunk matmul lhsT=X^T_chunk rhs=[X^T_chunk | qw^T_chunk]
     accumulating SC = [Gram | G_X] (bf16 inputs, fp32 accum)
  4. PE: M-fold: Mout = MT_bd.T @ SC = [v_n.x_j' | v_n.qw_l]
  5. DVE: sumsq_n = sum_j'(masked Mout); ACT: rsqrt via exp(-0.5*ln(x))
  6. scores scaled, transposed, masked softmax over sources (tiny ops)
  7. alphas folded through M (PE) -> B^T, H = B^T.T @ X in fp32r
  8. H PSUM -> SBUF -> one contiguous DMA out

Sharding: data-parallel over B*T = 2048 rows -> 8 cores x 256 rows.
"""

import numpy as np
import ml_dtypes

import concourse.bass as bass
import concourse.bacc as bacc
import concourse.mybir as mybir
from concourse import tile
from concourse.alu_op_type import AluOpType
from concourse.bass_utils import run_bass_kernel_spmd

L = 24
D = 2048
NUM_BLOCKS = 8
EPS = 1e-6
B, T = 2, 1024
N_CORES = 8

ROWS_PER_CORE = (B * T) // N_CORES  # 256
R = 5            # rows per batch
NJ = 25          # raw vectors per row: emb + 24 layer outputs
NS = 25          # sources per row: emb + (C_k1, C_k2, S_k) x 8 blocks
P = NJ * R       # 125 partitions per batch
NCHUNK = D // 128  # 16 d-chunks
CW = 152         # xt_sb column stride per chunk: 125 X^T + 24 qw + 3 pad
SCW = P + L      # 149 = gram + score columns
XF = D + 32      # padded row pitch (avoids flat-merged partition APs)
NEG = -1e30

f32 = mybir.dt.float32
f32r = mybir.dt.float32r
bf16 = mybir.dt.bfloat16


def _source_matrix():
    """M[n, j]: source n = sum_j M[n,j] * raw_j. Raw j=0 is emb, j=1+l is f_l.
    Sources: n=0 emb; n=1+3k+i (i=0,1,2) is C_{k,i+1} = f_{3k}+..+f_{3k+i}."""
    M = np.zeros((NS, NJ), dtype=np.float32)
    M[0, 0] = 1.0
    for k in range(NUM_BLOCKS):
        for i in range(3):
            n = 1 + 3 * k + i
            M[n, 1 + 3 * k : 1 + 3 * k + i + 1] = 1.0
    return M


def _valid_matrix():
    """valid[l, n]: which sources layer l attends over (block k=l//3, i=l%3):
    emb; S_k (n=3k+3) for k < l//3; partial C_{l//3, i} (n = 3*(l//3)+i) if i>0."""
    V = np.zeros((L, NS), dtype=bool)
    for l in range(L):
        kb, ii = l // 3, l % 3
        V[l, 0] = True
        for k in range(kb):
            V[l, 3 * k + 3] = True
        if ii > 0:
            V[l, 3 * kb + ii] = True
    return V


def _build_consts(queries, key_norm_weight):
    M = _source_matrix()
    valid = _valid_matrix()
    eye_r = np.eye(R, dtype=np.float32)

    qw = (queries * key_norm_weight[None, :]).astype(np.float32)  # [L, D]
    # qwT[p, c*24 + l] = qw[l, c*128 + p]
    qwT = np.ascontiguousarray(
        qw.reshape(L, NCHUNK, 128).transpose(2, 1, 0).reshape(128, NCHUNK * L)
    ).astype(ml_dtypes.bfloat16)

    # mtbd[(r,j),(r',n)] = (r==r') * M[n,j]   (lhsT of the M-fold matmul)
    mtbd = np.einsum("nj,ab->ajbn", M, eye_r).reshape(P, NS * R)
    mtbd = np.ascontiguousarray(mtbd).astype(ml_dtypes.bfloat16)
    # mbd[(r,n),(r',j)] = (r==r') * M[n,j]    (sumsq mask + B-fold lhsT)
    mbd = np.einsum("nj,ab->anbj", M, eye_r).reshape(NS * R, P)
    mbd = np.ascontiguousarray(mbd).astype(np.float32)
    # diagm[(r,n),(r',l)] = (r==r')
    diagm = np.einsum("ab,nl->anbl", eye_r, np.ones((NS, L), np.float32))
    diagm = np.ascontiguousarray(diagm.reshape(P, R * L)).astype(np.float32)
    # maskneg[l, (r,n)] = 0 if valid else NEG
    maskneg = np.where(valid[:, None, :], 0.0, NEG)  # [L, 1, NS] -> bcast r
    maskneg = np.broadcast_to(maskneg, (L, R, NS)).reshape(L, R * NS)
    maskneg = np.ascontiguousarray(maskneg).astype(np.float32)

    ident = np.eye(128, dtype=np.float32)
    return dict(qwT=qwT, mtbd=mtbd, mbd=mbd, diagm=diagm, maskneg=maskneg,
                ident=ident)


def _batch_starts():
    starts = [R * b for b in range(ROWS_PER_CORE // R)]  # 0..250
    if starts[-1] + R < ROWS_PER_CORE:
        starts.append(ROWS_PER_CORE - R)  # 251 (overlaps; identical rewrites)
    return starts


def build_kernel():
    nc = bacc.Bacc("TRN2", target_bir_lowering=False, debug=False)

    # host-transposed input: row-major [row, j, d] flattened. Declared f32r
    # (same bits as fp32) so the PE can consume it at full rate; walrus
    # requires fp32r matmul operands to be produced as fp32r. Row pitch is
    # padded to XF so the HBM-side read AP cannot flat-merge: unmerged per-row
    # descriptors spread the load DMA across all 16 SDMA engines (a merged
    # contiguous read is chunked into ~5 big descriptors = 5 engines only).
    loT = nc.dram_tensor("loT", [ROWS_PER_CORE * NJ, XF], f32r,
                         kind="ExternalInput").ap()
    qwT_d = nc.dram_tensor("qwT", [128, NCHUNK * L], bf16, kind="ExternalInput").ap()
    mtbd_d = nc.dram_tensor("mtbd", [P, NS * R], bf16, kind="ExternalInput").ap()
    mbd_d = nc.dram_tensor("mbd", [NS * R, P], f32, kind="ExternalInput").ap()
    diagm_d = nc.dram_tensor("diagm", [P, R * L], f32, kind="ExternalInput").ap()
    maskneg_d = nc.dram_tensor("maskneg", [L, R * NS], f32, kind="ExternalInput").ap()
    ident_d = nc.dram_tensor("ident", [128, 128], f32, kind="ExternalInput").ap()
    identr_d = nc.dram_tensor("identr", [128, 128], f32r, kind="ExternalInput").ap()
    # output [row, l, d] flattened; host transposes back to [l, row, d].
    # Row pitch padded to XF so the HBM-side write AP cannot flat-merge:
    # unmerged per-row descriptors spread the store DMA across all 16 SDMA
    # engines (a merged contiguous write is chunked into ~5 big descriptors
    # = 5 engines only, which was the measured bottleneck).
    outT = nc.dram_tensor("outT", [ROWS_PER_CORE * L, XF], f32,
                          kind="ExternalOutput").ap()

    with tile.TileContext(nc) as tc:
        with (
            tc.tile_pool(name="const", bufs=1) as const,
            tc.tile_pool(name="xpool", bufs=4) as xpool,
            tc.tile_pool(name="xtpool", bufs=3) as xtpool,
            tc.tile_pool(name="scpool", bufs=3) as scpool,
            tc.tile_pool(name="hpool", bufs=3) as hpool,
            tc.tile_pool(name="small", bufs=2) as small,
            tc.tile_pool(name="ps_xt", bufs=3, space=bass.MemorySpace.PSUM) as ps_xt,
            tc.tile_pool(name="ps_sc", bufs=1, space=bass.MemorySpace.PSUM) as ps_sc,
            tc.tile_pool(name="ps_m", bufs=1, space=bass.MemorySpace.PSUM) as ps_m,
            tc.tile_pool(name="ps_sm", bufs=1, space=bass.MemorySpace.PSUM) as ps_sm,
            tc.tile_pool(name="ps_h", bufs=2, space=bass.MemorySpace.PSUM) as ps_h,
        ):
            qwT = const.tile([128, NCHUNK * L], bf16)
            nc.sync.dma_start(qwT[:], qwT_d[:])
            mtbd = const.tile([P, NS * R], bf16)
            nc.sync.dma_start(mtbd[:], mtbd_d[:])
            mbd = const.tile([NS * R, P], f32)
            nc.sync.dma_start(mbd[:], mbd_d[:])
            diagm = const.tile([P, R * L], f32)
            nc.sync.dma_start(diagm[:], diagm_d[:])
            maskneg = const.tile([L, R * NS], f32)
            nc.sync.dma_start(maskneg[:], maskneg_d[:])
            ident = const.tile([128, 128], f32)
            nc.sync.dma_start(ident[:], ident_d[:])
            identr = const.tile([128, 128], f32r)
            nc.sync.dma_start(identr[:], identr_d[:])
            epsb = const.tile([P, 1], f32)
            nc.vector.memset(epsb[:], EPS)

            for row0 in _batch_starts():
                # ---- X = [emb; f_0..f_23] per row: one 1MB DMA, 16-way split
                X = xpool.tile([P, XF], f32r)
                nc.sync.dma_start(
                    X[:, 0:D], loT[row0 * NJ : row0 * NJ + P, 0:D]
                )

                # ---- X^T via PE transposes; bf16 copies into xt_sb
                xt_sb = xtpool.tile([128, NCHUNK * CW], bf16)
                xt3 = xt_sb.rearrange("p (c w) -> p c w", w=CW)
                nc.vector.tensor_copy(
                    xt3[:, :, P : P + L],
                    qwT.rearrange("p (c w) -> p c w", w=L),
                )
                for half in range(4):
                    xtp = ps_xt.tile([128, 512], f32r)
                    for cc in range(4):
                        c = 4 * half + cc
                        # fp32r dst needs an even innermost count: write 126
                        # cols via a zero-padded identity slice [I | 0]
                        nc.tensor.transpose(
                            xtp[:, 128 * cc : 128 * cc + P + 1],
                            X[:, 128 * c : 128 * (c + 1)],
                            identr[:P, : P + 1],
                        )
                    nc.scalar.copy(
                        xt3[:, 4 * half : 4 * half + 4, 0:P],
                        xtp.rearrange("p (cc w) -> p cc w", w=128)[:, :, 0:P],
                    )

                # ---- SC = [Gram | G_X] accumulated over d-chunks (bf16)
                SC = ps_sc.tile([P, 152], f32)
                for c in range(NCHUNK):
                    base = CW * c
                    nc.tensor.matmul(
                        SC[:, 0:SCW],
                        xt_sb[:, base : base + P],
                        xt_sb[:, base : base + SCW],
                        start=(c == 0),
                        stop=(c == NCHUNK - 1),
                    )
                SC_sb = scpool.tile([P, 152], bf16)
                nc.scalar.copy(SC_sb[:, 0:SCW], SC[:, 0:SCW])

                # ---- M-fold: Mout = [v_n . x_j' | v_n . qw_l]
                Mout = ps_m.tile([P, 152], f32)
                nc.tensor.matmul(
                    Mout[:, 0:SCW], mtbd[:], SC_sb[:, 0:SCW], start=True, stop=True
                )

                # ---- sumsq_n = sum over j' in source-set (masked row sum)
                junk = small.tile([P, P], f32)
                sumsq = small.tile([P, 1], f32)
                nc.vector.scalar_tensor_tensor(
                    out=junk[:],
                    in0=Mout[:, 0:P],
                    scalar=1.0,
                    in1=mbd[:],
                    op0=AluOpType.mult,
                    op1=AluOpType.mult,
                    accum_out=sumsq[:],
                )
                # rsqrt(mean+eps) = exp(-0.5 * ln(sumsq/D + eps))
                lnu = small.tile([P, 1], f32)
                nc.scalar.activation(
                    lnu[:], sumsq[:], mybir.ActivationFunctionType.Ln,
                    bias=epsb[:], scale=1.0 / D,
                )
                rsq = small.tile([P, 1], f32)
                nc.scalar.activation(
                    rsq[:], lnu[:], mybir.ActivationFunctionType.Exp, scale=-0.5
                )
                scoresR = small.tile([P, L], f32)
                nc.scalar.activation(
                    scoresR[:], Mout[:, P:SCW],
                    mybir.ActivationFunctionType.Copy, scale=rsq[:],
                )

                # ---- masked softmax over sources (free axis), per (r, l)
                scoreT = ps_sm.tile([L, P], f32, tag="sm")
                nc.tensor.transpose(scoreT[:], scoresR[:], ident[:P, :P])
                smask = small.tile([L, P], f32)
                nc.vector.tensor_add(smask[:], scoreT[:], maskneg[:])
                esc = small.tile([L, P], f32)
                nc.scalar.activation(
                    esc[:], smask[:], mybir.ActivationFunctionType.Exp
                )
                ssum = small.tile([L, R], f32)
                nc.vector.reduce_sum(
                    ssum[:],
                    esc.rearrange("p (r n) -> p r n", r=R),
                    axis=mybir.AxisListType.X,
                )
                rec = small.tile([L, R], f32)
                nc.vector.reciprocal(rec[:], ssum[:])
                alpha = small.tile([L, P], f32)
                nc.vector.tensor_tensor(
                    alpha.rearrange("p (r n) -> p r n", r=R),
                    esc.rearrange("p (r n) -> p r n", r=R),
                    rec.unsqueeze(2).broadcast_to([L, R, NS]),
                    AluOpType.mult,
                )

                # ---- fold alphas through M: B^T = M_bd.T @ alpha_bd
                alphaT = ps_sm.tile([P, L], f32, tag="sm")
                nc.tensor.transpose(alphaT[:], alpha[:], ident[:L, :L])
                abd = small.tile([P, R * L], f32)
                nc.vector.scalar_tensor_tensor(
                    out=abd.rearrange("p (r l) -> p r l", r=R),
                    in0=alphaT.unsqueeze(1).broadcast_to([P, R, L]),
                    scalar=1.0,
                    in1=diagm.rearrange("p (r l) -> p r l", r=R),
                    op0=AluOpType.mult,
                    op1=AluOpType.mult,
                )
                BT = ps_sm.tile([P, R * L], f32, tag="sm")
                nc.tensor.matmul(BT[:], mbd[:], abd[:], start=True, stop=True)
                btsb = small.tile([P, R * L], f32r)
                nc.scalar.copy(btsb[:], BT[:])

                # ---- H = B^T.T @ X  (fp32r, full-rate at N=512)
                H_sb = hpool.tile([R * L, XF], f32)
                for nb in range(4):
                    Hp = ps_h.tile([R * L, 512], f32)
                    nc.tensor.matmul(
                        Hp[:],
                        btsb[:],
                        X[:, 512 * nb : 512 * (nb + 1)],
                        start=True,
                        stop=True,
                    )
                    if nb % 2 == 0:
                        nc.scalar.copy(H_sb[:, 512 * nb : 512 * (nb + 1)], Hp[:])
                    else:
                        nc.vector.tensor_copy(
                            H_sb[:, 512 * nb : 512 * (nb + 1)], Hp[:]
                        )

                # out-DMA on the ACT HWDGE ring: keeps the sync ring free for
                # input prefetch (no head-of-line wait on H completion)
                nc.scalar.dma_start(
                    outT[row0 * L : row0 * L + R * L, 0:D], H_sb[:, 0:D]
                )

    # Pin Ln/Exp to the one table set containing both, so the compiled stream
    # has a single ACT table load instead of two reloads (~2.7us) per batch.
    # Set names/order (= act_func_set ids) are preserved; only the contents
    # steering the per-activation set choice are filtered.
    real_gat = bacc.get_activation_tables
    AF = mybir.ActivationFunctionType

    def gat_pinned(arch):
        out = {}
        for name, fns in real_gat(arch).items():
            if name == "natural_log_exp_and_others":
                out[name] = set(fns)
            else:
                out[name] = {f for f in fns if f not in (AF.Ln, AF.Exp)}
        return out

    bacc.get_activation_tables = gat_pinned
    try:
        nc.compile()
    finally:
        bacc.get_activation_tables = real_gat
    return nc


_NC_CACHE = None


def _prep_loT(layer_outputs, embedding):
    """[L,B,T,D]+[B,T,D] -> per-row stacks [B*T, 25, XF] (row-major,
    rows padded to the XF pitch)."""
    lo_flat = layer_outputs.reshape(L, B * T, D)
    emb_flat = embedding.reshape(B * T, D)
    loT = np.zeros((B * T, NJ, XF), dtype=np.float32)
    loT[:, 0, :D] = emb_flat
    loT[:, 1:, :D] = lo_flat.transpose(1, 0, 2)
    return loT


def kernel(layer_outputs, embedding, queries, key_norm_weight):
    global _NC_CACHE
    layer_outputs = np.asarray(layer_outputs, dtype=np.float32)
    embedding = np.asarray(embedding, dtype=np.float32)
    queries = np.asarray(queries, dtype=np.float32)
    key_norm_weight = np.asarray(key_norm_weight, dtype=np.float32)

    loT = _prep_loT(layer_outputs, embedding)
    consts = _build_consts(queries, key_norm_weight)

    if _NC_CACHE is None:
        _NC_CACHE = build_kernel()
    nc = _NC_CACHE

    in_maps = []
    for c in range(N_CORES):
        r0 = c * ROWS_PER_CORE
        in_maps.append({
            "loT": loT[r0 : r0 + ROWS_PER_CORE].reshape(ROWS_PER_CORE * NJ, XF),
            "qwT": consts["qwT"],
            "mtbd": consts["mtbd"],
            "mbd": consts["mbd"],
            "diagm": consts["diagm"],
            "maskneg": consts["maskneg"],
            "ident": consts["ident"],
            "identr": consts["ident"],
        })

    res = run_bass_kernel_spmd(nc, in_maps, core_ids=list(range(N_CORES)))

    full = np.empty((L, B * T, D), dtype=np.float32)
    for c in range(N_CORES):
        r0 = c * ROWS_PER_CORE
        outT = res.results[c]["outT"].reshape(ROWS_PER_CORE, L, XF)[:, :, :D]
        full[:, r0 : r0 + ROWS_PER_CORE, :] = outT.transpose(1, 0, 2)
    return full.reshape(L, B, T, D)



# revision 5
# speedup vs baseline: 2.2909x; 2.2909x over previous
"""Trainium2 Bass kernel for BlockAttnRes.compute_all_inputs.

Math: for each row (b,t), layer l attends over a small per-row source stack
(embedding, completed block sums S_k, and the running partial sum). Every
source is a prefix-sum of the 25 "raw" per-row vectors X = [emb, f_0..f_23],
i.e. sources V = M @ X for a constant 0/1 matrix M (25x25). Likewise the
output h_l = sum_n alpha_{l,n} v_n = (A M) @ X, and the score dots
v_n . qw_l = M @ (X @ qw^T). So the whole layer loop collapses into a few
small matmuls per row batch - no sequential layer recurrence on device.

Device layout: batches of R=5 rows; partition p = r*25 + j (r-major), j in
[0, 25) raw index, so P = 125 partitions. All streamed tensors are bf16
(rel-err budget 2e-2; bf16 keeps it ~1e-2 lower). The host ships BOTH
layouts of X so the device never transposes:
  - loT  [row, j, d]  row-major, one contiguous DMA per batch (H rhs)
  - xtT  [dmod128, batch, chunk, 152] d-major with the 24 qw^T columns and
    3 pad baked in after the 125 X^T columns (gram lhsT/rhs, score rhs)
Per batch:
  1. DMA X [125, D] bf16 and xt_sb [128, 16*152] bf16
  2. PE: per d-chunk matmul lhsT=xt[125] rhs=xt[149] accumulating
     SC = [Gram | G_X] (bf16 inputs, fp32 accum)
  3. PE: M-fold: Mout = MT_bd.T @ SC = [v_n.x_j' | v_n.qw_l]
  4. DVE: sumsq_n = sum_j'(masked Mout); ACT: rsqrt via exp(-0.5*ln(x))
  5. scores scaled, transposed, masked softmax over sources (tiny ops)
  6. alphas folded through M (PE) -> B^T (bf16), H = B^T.T @ X
  7. H PSUM -> SBUF bf16 -> one padded-pitch DMA out (bf16)

DMA engine spreading: the HWDGE splits a transfer's PARTITION dimension
evenly across SDMA engines, using the largest divisor of the partition
count <= 16. A 125-partition load therefore lands on only 5 engines (the
measured bottleneck of the f32 version), while 128 -> 16 and 120 -> 15.
So X is loaded as 128 partitions (3 garbage rows; loT padded by 3 rows)
and xtT is naturally 128; the 120-row output gets 15 engines.

Sharding: data-parallel over B*T = 2048 rows -> 8 cores x 256 rows.
"""

import numpy as np
import ml_dtypes

import concourse.bass as bass
import concourse.bacc as bacc
import concourse.mybir as mybir
from concourse import tile
from concourse.alu_op_type import AluOpType
from concourse.bass_utils import run_bass_kernel_spmd

L = 24
D = 2048
NUM_BLOCKS = 8
EPS = 1e-6
B, T = 2, 1024
N_CORES = 8

ROWS_PER_CORE = (B * T) // N_CORES  # 256
R = 5            # rows per batch
NJ = 25          # raw vectors per row: emb + 24 layer outputs
NS = 25          # sources per row: emb + (C_k1, C_k2, S_k) x 8 blocks
P = NJ * R       # 125 partitions per batch
NCHUNK = D // 128  # 16 d-chunks
CW = 152         # xt column stride per chunk: 125 X^T + 24 qw + 3 pad
SCW = P + L      # 149 = gram + score columns
XF = D + 32      # padded row pitch (avoids flat-merged partition APs)
NEG = -1e30

f32 = mybir.dt.float32
bf16 = mybir.dt.bfloat16
bfnp = ml_dtypes.bfloat16


def _source_matrix():
    """M[n, j]: source n = sum_j M[n,j] * raw_j. Raw j=0 is emb, j=1+l is f_l.
    Sources: n=0 emb; n=1+3k+i (i=0,1,2) is C_{k,i+1} = f_{3k}+..+f_{3k+i}."""
    M = np.zeros((NS, NJ), dtype=np.float32)
    M[0, 0] = 1.0
    for k in range(NUM_BLOCKS):
        for i in range(3):
            n = 1 + 3 * k + i
            M[n, 1 + 3 * k : 1 + 3 * k + i + 1] = 1.0
    return M


def _valid_matrix():
    """valid[l, n]: which sources layer l attends over (block k=l//3, i=l%3):
    emb; S_k (n=3k+3) for k < l//3; partial C_{l//3, i} (n = 3*(l//3)+i) if i>0."""
    V = np.zeros((L, NS), dtype=bool)
    for l in range(L):
        kb, ii = l // 3, l % 3
        V[l, 0] = True
        for k in range(kb):
            V[l, 3 * k + 3] = True
        if ii > 0:
            V[l, 3 * kb + ii] = True
    return V


def _build_consts(queries, key_norm_weight):
    M = _source_matrix()
    valid = _valid_matrix()
    eye_r = np.eye(R, dtype=np.float32)

    qw = (queries * key_norm_weight[None, :]).astype(np.float32)  # [L, D]
    # qwT[p, c*24 + l] = qw[l, c*128 + p]
    qwT = np.ascontiguousarray(
        qw.reshape(L, NCHUNK, 128).transpose(2, 1, 0).reshape(128, NCHUNK * L)
    ).astype(bfnp)

    # mtbd[(r,j),(r',n)] = (r==r') * M[n,j]   (lhsT of the M-fold matmul)
    mtbd = np.einsum("nj,ab->ajbn", M, eye_r).reshape(P, NS * R)
    mtbd = np.ascontiguousarray(mtbd).astype(bfnp)
    # mbd[(r,n),(r',j)] = (r==r') * M[n,j]    (sumsq mask + B-fold lhsT)
    mbd = np.einsum("nj,ab->anbj", M, eye_r).reshape(NS * R, P)
    mbd = np.ascontiguousarray(mbd).astype(np.float32)
    mbdb = mbd.astype(bfnp)
    # diagm[(r,n),(r',l)] = (r==r')
    diagm = np.einsum("ab,nl->anbl", eye_r, np.ones((NS, L), np.float32))
    diagm = np.ascontiguousarray(diagm.reshape(P, R * L)).astype(np.float32)
    # maskneg[l, (r,n)] = 0 if valid else NEG
    maskneg = np.where(valid[:, None, :], 0.0, NEG)  # [L, 1, NS] -> bcast r
    maskneg = np.broadcast_to(maskneg, (L, R, NS)).reshape(L, R * NS)
    maskneg = np.ascontiguousarray(maskneg).astype(np.float32)

    ident = np.eye(128, dtype=np.float32)
    return dict(qwT=qwT, mtbd=mtbd, mbd=mbd, mbdb=mbdb, diagm=diagm,
                maskneg=maskneg, ident=ident)


def _batch_starts():
    starts = [R * b for b in range(ROWS_PER_CORE // R)]  # 0..250
    if starts[-1] + R < ROWS_PER_CORE:
        starts.append(ROWS_PER_CORE - R)  # 251 (overlaps; identical rewrites)
    return starts


NBATCH = len(_batch_starts())  # 52


def build_kernel():
    nc = bacc.Bacc("TRN2", target_bir_lowering=False, debug=False)

    # host-transposed inputs, bf16. loT has 3 pad rows so every batch can
    # load 128 partitions (16-engine DMA split) without running off the end.
    loT = nc.dram_tensor("loT", [ROWS_PER_CORE * NJ + 3, XF], bf16,
                         kind="ExternalInput").ap()
    xtT_d = nc.dram_tensor("xtT", [128, NBATCH * NCHUNK * CW], bf16,
                           kind="ExternalInput").ap()
    mtbd_d = nc.dram_tensor("mtbd", [P, NS * R], bf16, kind="ExternalInput").ap()
    mbd_d = nc.dram_tensor("mbd", [NS * R, P], f32, kind="ExternalInput").ap()
    mbdb_d = nc.dram_tensor("mbdb", [NS * R, P], bf16, kind="ExternalInput").ap()
    diagm_d = nc.dram_tensor("diagm", [P, R * L], f32, kind="ExternalInput").ap()
    maskneg_d = nc.dram_tensor("maskneg", [L, R * NS], f32, kind="ExternalInput").ap()
    ident_d = nc.dram_tensor("ident", [128, 128], f32, kind="ExternalInput").ap()
    # output [row, l, d] flattened, bf16, padded pitch; host strips the pad
    # and transposes back to [l, row, d]
    outT = nc.dram_tensor("outT", [ROWS_PER_CORE * L, XF], bf16,
                          kind="ExternalOutput").ap()

    with tile.TileContext(nc) as tc:
        with (
            tc.tile_pool(name="const", bufs=1) as const,
            tc.tile_pool(name="xpool", bufs=4) as xpool,
            tc.tile_pool(name="xtpool", bufs=4) as xtpool,
            tc.tile_pool(name="scpool", bufs=3) as scpool,
            tc.tile_pool(name="hpool", bufs=4) as hpool,
            tc.tile_pool(name="small", bufs=2) as small,
            tc.tile_pool(name="ps_sc", bufs=2, space=bass.MemorySpace.PSUM) as ps_sc,
            tc.tile_pool(name="ps_m", bufs=2, space=bass.MemorySpace.PSUM) as ps_m,
            tc.tile_pool(name="ps_sm", bufs=1, space=bass.MemorySpace.PSUM) as ps_sm,
            tc.tile_pool(name="ps_h", bufs=2, space=bass.MemorySpace.PSUM) as ps_h,
        ):
            mtbd = const.tile([P, NS * R], bf16)
            nc.sync.dma_start(mtbd[:], mtbd_d[:])
            mbd = const.tile([NS * R, P], f32)
            nc.sync.dma_start(mbd[:], mbd_d[:])
            mbdb = const.tile([NS * R, P], bf16)
            nc.sync.dma_start(mbdb[:], mbdb_d[:])
            diagm = const.tile([P, R * L], f32)
            nc.sync.dma_start(diagm[:], diagm_d[:])
            maskneg = const.tile([L, R * NS], f32)
            nc.sync.dma_start(maskneg[:], maskneg_d[:])
            ident = const.tile([128, 128], f32)
            nc.sync.dma_start(ident[:], ident_d[:])
            epsb = const.tile([P, 1], f32)
            nc.vector.memset(epsb[:], EPS)

            for bi, row0 in enumerate(_batch_starts()):
                # ---- X = [emb; f_0..f_23] per row (H rhs). Loaded as 128
                # partitions (last 3 are unused junk) for a 16-engine split.
                X = xpool.tile([128, XF], bf16)
                nc.sync.dma_start(
                    X[:, 0:D], loT[row0 * NJ : row0 * NJ + 128, 0:D]
                )
                # ---- X^T with qw columns baked in, 128 descriptors
                xt_sb = xtpool.tile([128, NCHUNK * CW], bf16)
                nc.sync.dma_start(
                    xt_sb[:],
                    xtT_d[:, bi * NCHUNK * CW : (bi + 1) * NCHUNK * CW],
                )

                # ---- SC = [Gram | G_X] accumulated over d-chunks (bf16)
                SC = ps_sc.tile([P, 152], f32)
                for c in range(NCHUNK):
                    base = CW * c
                    nc.tensor.matmul(
                        SC[:, 0:SCW],
                        xt_sb[:, base : base + P],
                        xt_sb[:, base : base + SCW],
                        start=(c == 0),
                        stop=(c == NCHUNK - 1),
                    )
                SC_sb = scpool.tile([P, 152], bf16)
                nc.scalar.copy(SC_sb[:, 0:SCW], SC[:, 0:SCW])

                # ---- M-fold: Mout = [v_n . x_j' | v_n . qw_l]
                Mout = ps_m.tile([P, 152], f32)
                nc.tensor.matmul(
                    Mout[:, 0:SCW], mtbd[:], SC_sb[:, 0:SCW], start=True, stop=True
                )

                # ---- sumsq_n = sum over j' in source-set (masked row sum)
                junk = small.tile([P, P], f32)
                sumsq = small.tile([P, 1], f32)
                nc.vector.scalar_tensor_tensor(
                    out=junk[:],
                    in0=Mout[:, 0:P],
                    scalar=1.0,
                    in1=mbd[:],
                    op0=AluOpType.mult,
                    op1=AluOpType.mult,
                    accum_out=sumsq[:],
                )
                # rsqrt(mean+eps) = exp(-0.5 * ln(sumsq/D + eps))
                lnu = small.tile([P, 1], f32)
                nc.scalar.activation(
                    lnu[:], sumsq[:], mybir.ActivationFunctionType.Ln,
                    bias=epsb[:], scale=1.0 / D,
                )
                rsq = small.tile([P, 1], f32)
                nc.scalar.activation(
                    rsq[:], lnu[:], mybir.ActivationFunctionType.Exp, scale=-0.5
                )
                scoresR = small.tile([P, L], f32)
                nc.scalar.activation(
                    scoresR[:], Mout[:, P:SCW],
                    mybir.ActivationFunctionType.Copy, scale=rsq[:],
                )

                # ---- masked softmax over sources (free axis), per (r, l)
                scoreT = ps_sm.tile([L, P], f32, tag="sm")
                nc.tensor.transpose(scoreT[:], scoresR[:], ident[:P, :P])
                smask = small.tile([L, P], f32)
                nc.vector.tensor_add(smask[:], scoreT[:], maskneg[:])
                esc = small.tile([L, P], f32)
                nc.scalar.activation(
                    esc[:], smask[:], mybir.ActivationFunctionType.Exp
                )
                ssum = small.tile([L, R], f32)
                nc.vector.reduce_sum(
                    ssum[:],
                    esc.rearrange("p (r n) -> p r n", r=R),
                    axis=mybir.AxisListType.X,
                )
                rec = small.tile([L, R], f32)
                nc.vector.reciprocal(rec[:], ssum[:])
                alpha = small.tile([L, P], f32)
                nc.vector.tensor_tensor(
                    alpha.rearrange("p (r n) -> p r n", r=R),
                    esc.rearrange("p (r n) -> p r n", r=R),
                    rec.unsqueeze(2).broadcast_to([L, R, NS]),
                    AluOpType.mult,
                )

                # ---- fold alphas through M: B^T = M_bd.T @ alpha_bd
                alphaT = ps_sm.tile([P, L], f32, tag="sm")
                nc.tensor.transpose(alphaT[:], alpha[:], ident[:L, :L])
                abd = small.tile([P, R * L], bf16)
                nc.vector.scalar_tensor_tensor(
                    out=abd.rearrange("p (r l) -> p r l", r=R),
                    in0=alphaT.unsqueeze(1).broadcast_to([P, R, L]),
                    scalar=1.0,
                    in1=diagm.rearrange("p (r l) -> p r l", r=R),
                    op0=AluOpType.mult,
                    op1=AluOpType.mult,
                )
                BT = ps_sm.tile([P, R * L], f32, tag="sm")
                nc.tensor.matmul(BT[:], mbdb[:], abd[:], start=True, stop=True)
                btsb = small.tile([P, R * L], bf16)
                nc.scalar.copy(btsb[:], BT[:])

                # ---- H = B^T.T @ X  (bf16 in, fp32 accum)
                H_sb = hpool.tile([R * L, XF], bf16)
                for nb in range(4):
                    Hp = ps_h.tile([R * L, 512], f32)
                    nc.tensor.matmul(
                        Hp[:],
                        btsb[:],
                        X[0:P, 512 * nb : 512 * (nb + 1)],
                        start=True,
                        stop=True,
                    )
                    if nb % 2 == 0:
                        nc.scalar.copy(H_sb[:, 512 * nb : 512 * (nb + 1)], Hp[:])
                    else:
                        nc.vector.tensor_copy(
                            H_sb[:, 512 * nb : 512 * (nb + 1)], Hp[:]
                        )

                # out-DMA on the ACT HWDGE ring: keeps the sync ring free for
                # input prefetch (no head-of-line wait on H completion)
                nc.scalar.dma_start(
                    outT[row0 * L : row0 * L + R * L, 0:D], H_sb[:, 0:D]
                )

    # Pin Ln/Exp to the one table set containing both, so the compiled stream
    # has a single ACT table load instead of two reloads (~2.7us) per batch.
    # Set names/order (= act_func_set ids) are preserved; only the contents
    # steering the per-activation set choice are filtered.
    real_gat = bacc.get_activation_tables
    AF = mybir.ActivationFunctionType

    def gat_pinned(arch):
        out = {}
        for name, fns in real_gat(arch).items():
            if name == "natural_log_exp_and_others":
                out[name] = set(fns)
            else:
                out[name] = {f for f in fns if f not in (AF.Ln, AF.Exp)}
        return out

    bacc.get_activation_tables = gat_pinned
    try:
        nc.compile()
    finally:
        bacc.get_activation_tables = real_gat
    return nc


_NC_CACHE = None


def _prep_loT(layer_outputs, embedding):
    """[L,B,T,D]+[B,T,D] -> per-row stacks [B*T, 25, XF] bf16 (row-major,
    rows padded to the XF pitch)."""
    lo_flat = layer_outputs.reshape(L, B * T, D)
    emb_flat = embedding.reshape(B * T, D)
    loT = np.zeros((B * T, NJ, XF), dtype=bfnp)
    loT[:, 0, :D] = emb_flat.astype(bfnp)
    loT[:, 1:, :D] = lo_flat.transpose(1, 0, 2).astype(bfnp)
    return loT


def _prep_xt(loT_core, qwT):
    """Per-core d-major stream: [128, NBATCH, NCHUNK, CW] flattened, where
    xt[p, b, c, j] = X_b[j, c*128+p] for j<125, qw^T columns at 125:149."""
    out = np.zeros((128, NBATCH, NCHUNK, CW), dtype=bfnp)
    # batches 0..50 tile rows 0..254 contiguously
    x51 = loT_core[: 51 * R, :, :D].reshape(51, P, NCHUNK, 128)
    out[:, :51, :, 0:P] = x51.transpose(3, 0, 2, 1)
    # final overlap batch starts at row 251
    row0 = _batch_starts()[-1]
    xl = loT_core[row0 : row0 + R, :, :D].reshape(P, NCHUNK, 128)
    out[:, 51, :, 0:P] = xl.transpose(2, 1, 0)
    out[:, :, :, P:SCW] = qwT.reshape(128, NCHUNK, L)[:, None, :, :]
    return np.ascontiguousarray(out.reshape(128, NBATCH * NCHUNK * CW))


def _make_in_maps(layer_outputs, embedding, queries, key_norm_weight):
    loT = _prep_loT(layer_outputs, embedding)
    consts = _build_consts(queries, key_norm_weight)
    in_maps = []
    pad3 = np.zeros((3, XF), dtype=bfnp)
    for c in range(N_CORES):
        r0 = c * ROWS_PER_CORE
        core_loT = loT[r0 : r0 + ROWS_PER_CORE]
        in_maps.append({
            "loT": np.concatenate(
                [core_loT.reshape(ROWS_PER_CORE * NJ, XF), pad3], axis=0
            ),
            "xtT": _prep_xt(core_loT, consts["qwT"]),
            "mtbd": consts["mtbd"],
            "mbd": consts["mbd"],
            "mbdb": consts["mbdb"],
            "diagm": consts["diagm"],
            "maskneg": consts["maskneg"],
            "ident": consts["ident"],
        })
    return in_maps


def kernel(layer_outputs, embedding, queries, key_norm_weight):
    global _NC_CACHE
    layer_outputs = np.asarray(layer_outputs, dtype=np.float32)
    embedding = np.asarray(embedding, dtype=np.float32)
    queries = np.asarray(queries, dtype=np.float32)
    key_norm_weight = np.asarray(key_norm_weight, dtype=np.float32)

    in_maps = _make_in_maps(layer_outputs, embedding, queries, key_norm_weight)

    if _NC_CACHE is None:
        _NC_CACHE = build_kernel()
    nc = _NC_CACHE

    res = run_bass_kernel_spmd(nc, in_maps, core_ids=list(range(N_CORES)))

    full = np.empty((L, B * T, D), dtype=np.float32)
    for c in range(N_CORES):
        r0 = c * ROWS_PER_CORE
        outT = res.results[c]["outT"].reshape(ROWS_PER_CORE, L, XF)[:, :, :D]
        full[:, r0 : r0 + ROWS_PER_CORE, :] = (
            outT.astype(np.float32).transpose(1, 0, 2)
        )
    return full.reshape(L, B, T, D)


# revision 7
# speedup vs baseline: 2.9349x; 1.2811x over previous
"""Trainium2 Bass kernel for BlockAttnRes.compute_all_inputs.

Math: for each row (b,t), layer l attends over a small per-row source stack
(embedding, completed block sums S_k, and the running partial sum). Every
source is a prefix-sum of the 25 "raw" per-row vectors X = [emb, f_0..f_23],
i.e. sources V = M @ X for a constant 0/1 matrix M (25x25). Likewise the
output h_l = sum_n alpha_{l,n} v_n = (A M) @ X, and the score dots
v_n . qw_l = M @ (X @ qw^T). So the whole layer loop collapses into a few
small matmuls per row batch - no sequential layer recurrence on device.

Device layout: batches of R=5 rows; partition p = r*25 + j (r-major), j in
[0, 25) raw index, so P = 125 partitions. All streamed tensors are bf16
(rel-err budget 2e-2; bf16 keeps it ~1e-2 lower). The host ships BOTH
layouts of X so the device never transposes:
  - loT  [row, j, d]  row-major, one contiguous DMA per batch (H rhs)
  - xtT  [dmod128, batch, chunk, 152] d-major with the 24 qw^T columns and
    3 pad baked in after the 125 X^T columns (gram lhsT/rhs, score rhs)
Per batch:
  1. DMA X [125, D] bf16 and xt_sb [128, 16*152] bf16
  2. PE: per d-chunk matmul lhsT=xt[125] rhs=xt[149] accumulating
     SC = [Gram | G_X] (bf16 inputs, fp32 accum)
  3. PE: M-fold: Mout = MT_bd.T @ SC = [v_n.x_j' | v_n.qw_l]
  4. DVE: sumsq_n = sum_j'(masked Mout); ACT: rsqrt via exp(-0.5*ln(x))
  5. scores scaled, transposed, masked softmax over sources (tiny ops)
  6. alphas folded through M (PE) -> B^T (bf16), H = B^T.T @ X
  7. H PSUM -> SBUF bf16 -> one padded-pitch DMA out (bf16)

DMA engine spreading: the HWDGE splits a transfer's PARTITION dimension
evenly across SDMA engines, using the largest divisor of the partition
count <= 16. A 125-partition load therefore lands on only 5 engines (the
measured bottleneck of the f32 version), while 128 -> 16 and 120 -> 15.
So X is loaded as 128 partitions (3 garbage rows; loT padded by 3 rows)
and xtT is naturally 128; the 120-row output gets 15 engines.

Sharding: data-parallel over B*T = 2048 rows -> 8 cores x 256 rows.
"""

import numpy as np
import ml_dtypes

import concourse.bass as bass
import concourse.bacc as bacc
import concourse.mybir as mybir
from concourse import tile
from concourse.alu_op_type import AluOpType
from concourse.bass_utils import run_bass_kernel_spmd

L = 24
D = 2048
NUM_BLOCKS = 8
EPS = 1e-6
B, T = 2, 1024
N_CORES = 8

ROWS_PER_CORE = (B * T) // N_CORES  # 256
R = 5            # rows per batch
NJ = 25          # raw vectors per row: emb + 24 layer outputs
NS = 25          # sources per row: emb + (C_k1, C_k2, S_k) x 8 blocks
P = NJ * R       # 125 partitions per batch
NCHUNK = D // 128  # 16 d-chunks
CW = 152         # xt column stride per chunk: 125 X^T + 24 qw + 3 pad
SCW = P + L      # 149 = gram + score columns
XF = D + 32      # padded row pitch (avoids flat-merged partition APs)
NEG = -1e30

f32 = mybir.dt.float32
bf16 = mybir.dt.bfloat16
bfnp = ml_dtypes.bfloat16


def _source_matrix():
    """M[n, j]: source n = sum_j M[n,j] * raw_j. Raw j=0 is emb, j=1+l is f_l.
    Sources: n=0 emb; n=1+3k+i (i=0,1,2) is C_{k,i+1} = f_{3k}+..+f_{3k+i}."""
    M = np.zeros((NS, NJ), dtype=np.float32)
    M[0, 0] = 1.0
    for k in range(NUM_BLOCKS):
        for i in range(3):
            n = 1 + 3 * k + i
            M[n, 1 + 3 * k : 1 + 3 * k + i + 1] = 1.0
    return M


def _valid_matrix():
    """valid[l, n]: which sources layer l attends over (block k=l//3, i=l%3):
    emb; S_k (n=3k+3) for k < l//3; partial C_{l//3, i} (n = 3*(l//3)+i) if i>0."""
    V = np.zeros((L, NS), dtype=bool)
    for l in range(L):
        kb, ii = l // 3, l % 3
        V[l, 0] = True
        for k in range(kb):
            V[l, 3 * k + 3] = True
        if ii > 0:
            V[l, 3 * kb + ii] = True
    return V


def _build_consts(queries, key_norm_weight):
    M = _source_matrix()
    valid = _valid_matrix()
    eye_r = np.eye(R, dtype=np.float32)

    qw = (queries * key_norm_weight[None, :]).astype(np.float32)  # [L, D]
    # qwT[p, c*24 + l] = qw[l, c*128 + p]
    qwT = np.ascontiguousarray(
        qw.reshape(L, NCHUNK, 128).transpose(2, 1, 0).reshape(128, NCHUNK * L)
    ).astype(bfnp)

    # mtbd[(r,j),(r',n)] = (r==r') * M[n,j]   (lhsT of the M-fold matmul)
    mtbd = np.einsum("nj,ab->ajbn", M, eye_r).reshape(P, NS * R)
    mtbd = np.ascontiguousarray(mtbd).astype(bfnp)
    # mbd[(r,n),(r',j)] = (r==r') * M[n,j]    (sumsq mask + B-fold lhsT)
    mbd = np.einsum("nj,ab->anbj", M, eye_r).reshape(NS * R, P)
    mbd = np.ascontiguousarray(mbd).astype(np.float32)
    mbdb = mbd.astype(bfnp)
    # diagm[(r,n),(r',l)] = (r==r')
    diagm = np.einsum("ab,nl->anbl", eye_r, np.ones((NS, L), np.float32))
    diagm = np.ascontiguousarray(diagm.reshape(P, R * L)).astype(np.float32)
    # maskneg[l, (r,n)] = 0 if valid else NEG
    maskneg = np.where(valid[:, None, :], 0.0, NEG)  # [L, 1, NS] -> bcast r
    maskneg = np.broadcast_to(maskneg, (L, R, NS)).reshape(L, R * NS)
    maskneg = np.ascontiguousarray(maskneg).astype(np.float32)

    ident = np.eye(128, dtype=np.float32)
    return dict(qwT=qwT, mtbd=mtbd, mbd=mbd, mbdb=mbdb, diagm=diagm,
                maskneg=maskneg, ident=ident)


def _batch_starts():
    starts = [R * b for b in range(ROWS_PER_CORE // R)]  # 0..250
    if starts[-1] + R < ROWS_PER_CORE:
        starts.append(ROWS_PER_CORE - R)  # 251 (overlaps; identical rewrites)
    return starts


NBATCH = len(_batch_starts())  # 52


def build_kernel():
    nc = bacc.Bacc("TRN2", target_bir_lowering=False, debug=False)

    # host-transposed inputs, bf16. loT has 3 pad rows so every batch can
    # load 128 partitions (16-engine DMA split) without running off the end.
    loT = nc.dram_tensor("loT", [ROWS_PER_CORE * NJ + 3, XF], bf16,
                         kind="ExternalInput").ap()
    xtT_d = nc.dram_tensor("xtT", [128, NBATCH * NCHUNK * CW], bf16,
                           kind="ExternalInput").ap()
    mtbd_d = nc.dram_tensor("mtbd", [P, NS * R], bf16, kind="ExternalInput").ap()
    mbd_d = nc.dram_tensor("mbd", [NS * R, P], f32, kind="ExternalInput").ap()
    mbdb_d = nc.dram_tensor("mbdb", [NS * R, P], bf16, kind="ExternalInput").ap()
    diagm_d = nc.dram_tensor("diagm", [P, R * L], f32, kind="ExternalInput").ap()
    maskneg_d = nc.dram_tensor("maskneg", [L, R * NS], f32, kind="ExternalInput").ap()
    ident_d = nc.dram_tensor("ident", [128, 128], f32, kind="ExternalInput").ap()
    # output [row, l, d] flattened, bf16, padded pitch; host strips the pad
    # and transposes back to [l, row, d]
    outT = nc.dram_tensor("outT", [ROWS_PER_CORE * L, XF], bf16,
                          kind="ExternalOutput").ap()

    with tile.TileContext(nc) as tc:
        with (
            tc.tile_pool(name="const", bufs=1) as const,
            tc.tile_pool(name="xpool", bufs=6) as xpool,
            tc.tile_pool(name="xtpool", bufs=6) as xtpool,
            tc.tile_pool(name="scpool", bufs=3) as scpool,
            tc.tile_pool(name="hpool", bufs=4) as hpool,
            tc.tile_pool(name="small", bufs=3) as small,
            tc.tile_pool(name="ps_sc", bufs=2, space=bass.MemorySpace.PSUM) as ps_sc,
            tc.tile_pool(name="ps_m", bufs=1, space=bass.MemorySpace.PSUM) as ps_m,
            tc.tile_pool(name="ps_sm", bufs=2, space=bass.MemorySpace.PSUM) as ps_sm,
            tc.tile_pool(name="ps_h", bufs=3, space=bass.MemorySpace.PSUM) as ps_h,
        ):
            mtbd = const.tile([P, NS * R], bf16)
            nc.sync.dma_start(mtbd[:], mtbd_d[:])
            mbd = const.tile([NS * R, P], f32)
            nc.sync.dma_start(mbd[:], mbd_d[:])
            mbdb = const.tile([NS * R, P], bf16)
            nc.sync.dma_start(mbdb[:], mbdb_d[:])
            diagm = const.tile([P, R * L], f32)
            nc.sync.dma_start(diagm[:], diagm_d[:])
            maskneg = const.tile([L, R * NS], f32)
            nc.sync.dma_start(maskneg[:], maskneg_d[:])
            ident = const.tile([128, 128], f32)
            nc.sync.dma_start(ident[:], ident_d[:])
            epsb = const.tile([P, 1], f32)
            nc.vector.memset(epsb[:], EPS)

            for bi, row0 in enumerate(_batch_starts()):
                # ---- X = [emb; f_0..f_23] per row (H rhs). Loaded as 128
                # partitions (last 3 are unused junk) for a 16-engine split.
                X = xpool.tile([128, XF], bf16)
                nc.sync.dma_start(
                    X[:, 0:D], loT[row0 * NJ : row0 * NJ + 128, 0:D]
                )
                # ---- X^T with qw columns baked in, 128 descriptors
                xt_sb = xtpool.tile([128, NCHUNK * CW], bf16)
                nc.sync.dma_start(
                    xt_sb[:],
                    xtT_d[:, bi * NCHUNK * CW : (bi + 1) * NCHUNK * CW],
                )

                # ---- SC = [Gram | G_X] accumulated over d-chunks (bf16)
                SC = ps_sc.tile([P, 152], f32)
                for c in range(NCHUNK):
                    base = CW * c
                    nc.tensor.matmul(
                        SC[:, 0:SCW],
                        xt_sb[:, base : base + P],
                        xt_sb[:, base : base + SCW],
                        start=(c == 0),
                        stop=(c == NCHUNK - 1),
                    )
                SC_sb = scpool.tile([P, 152], bf16)
                nc.scalar.copy(SC_sb[:, 0:SCW], SC[:, 0:SCW])

                # ---- M-fold: Mout = [v_n . x_j' | v_n . qw_l]
                Mout = ps_m.tile([P, 152], f32)
                nc.tensor.matmul(
                    Mout[:, 0:SCW], mtbd[:], SC_sb[:, 0:SCW], start=True, stop=True
                )

                # ---- sumsq_n = sum over j' in source-set (masked row sum)
                junk = small.tile([P, P], f32)
                sumsq = small.tile([P, 1], f32)
                nc.vector.scalar_tensor_tensor(
                    out=junk[:],
                    in0=Mout[:, 0:P],
                    scalar=1.0,
                    in1=mbd[:],
                    op0=AluOpType.mult,
                    op1=AluOpType.mult,
                    accum_out=sumsq[:],
                )
                # rsqrt(mean+eps) = exp(-0.5 * ln(sumsq/D + eps))
                lnu = small.tile([P, 1], f32)
                nc.scalar.activation(
                    lnu[:], sumsq[:], mybir.ActivationFunctionType.Ln,
                    bias=epsb[:], scale=1.0 / D,
                )
                rsq = small.tile([P, 1], f32)
                nc.scalar.activation(
                    rsq[:], lnu[:], mybir.ActivationFunctionType.Exp, scale=-0.5
                )
                scoresR = small.tile([P, L], f32)
                nc.scalar.activation(
                    scoresR[:], Mout[:, P:SCW],
                    mybir.ActivationFunctionType.Copy, scale=rsq[:],
                )

                # ---- masked softmax over sources (free axis), per (r, l)
                scoreT = ps_sm.tile([L, P], f32, tag="sm")
                nc.tensor.transpose(scoreT[:], scoresR[:], ident[:P, :P])
                smask = small.tile([L, P], f32)
                nc.vector.tensor_add(smask[:], scoreT[:], maskneg[:])
                esc = small.tile([L, P], f32)
                nc.scalar.activation(
                    esc[:], smask[:], mybir.ActivationFunctionType.Exp
                )
                ssum = small.tile([L, R], f32)
                nc.vector.reduce_sum(
                    ssum[:],
                    esc.rearrange("p (r n) -> p r n", r=R),
                    axis=mybir.AxisListType.X,
                )
                rec = small.tile([L, R], f32)
                nc.vector.reciprocal(rec[:], ssum[:])
                alpha = small.tile([L, P], f32)
                nc.vector.tensor_tensor(
                    alpha.rearrange("p (r n) -> p r n", r=R),
                    esc.rearrange("p (r n) -> p r n", r=R),
                    rec.unsqueeze(2).broadcast_to([L, R, NS]),
                    AluOpType.mult,
                )

                # ---- fold alphas through M: B^T = M_bd.T @ alpha_bd
                alphaT = ps_sm.tile([P, L], f32, tag="sm")
                nc.tensor.transpose(alphaT[:], alpha[:], ident[:L, :L])
                abd = small.tile([P, R * L], bf16)
                nc.vector.scalar_tensor_tensor(
                    out=abd.rearrange("p (r l) -> p r l", r=R),
                    in0=alphaT.unsqueeze(1).broadcast_to([P, R, L]),
                    scalar=1.0,
                    in1=diagm.rearrange("p (r l) -> p r l", r=R),
                    op0=AluOpType.mult,
                    op1=AluOpType.mult,
                )
                BT = ps_sm.tile([P, R * L], f32, tag="sm")
                nc.tensor.matmul(BT[:], mbdb[:], abd[:], start=True, stop=True)
                btsb = small.tile([P, R * L], bf16)
                nc.scalar.copy(btsb[:], BT[:])

                # ---- H = B^T.T @ X  (bf16 in, fp32 accum)
                H_sb = hpool.tile([R * L, XF], bf16)
                for nb in range(4):
                    Hp = ps_h.tile([R * L, 512], f32)
                    nc.tensor.matmul(
                        Hp[:],
                        btsb[:],
                        X[0:P, 512 * nb : 512 * (nb + 1)],
                        start=True,
                        stop=True,
                    )
                    if nb % 2 == 0:
                        nc.scalar.copy(H_sb[:, 512 * nb : 512 * (nb + 1)], Hp[:])
                    else:
                        nc.vector.tensor_copy(
                            H_sb[:, 512 * nb : 512 * (nb + 1)], Hp[:]
                        )

                # out-DMA on the idle GpSimd HWDGE ring: keeps the sync ring
                # free for input prefetch and the ACT sequencer free for the
                # PSUM->SBUF copies (dispatch costs ~850ns of engine time)
                nc.gpsimd.dma_start(
                    outT[row0 * L : row0 * L + R * L, 0:D], H_sb[:, 0:D]
                )

    # Pin Ln/Exp to the one table set containing both, so the compiled stream
    # has a single ACT table load instead of two reloads (~2.7us) per batch.
    # Set names/order (= act_func_set ids) are preserved; only the contents
    # steering the per-activation set choice are filtered.
    real_gat = bacc.get_activation_tables
    AF = mybir.ActivationFunctionType

    def gat_pinned(arch):
        out = {}
        for name, fns in real_gat(arch).items():
            if name == "natural_log_exp_and_others":
                out[name] = set(fns)
            else:
                out[name] = {f for f in fns if f not in (AF.Ln, AF.Exp)}
        return out

    bacc.get_activation_tables = gat_pinned
    try:
        nc.compile()
    finally:
        bacc.get_activation_tables = real_gat
    return nc


_NC_CACHE = None


def _prep_loT(layer_outputs, embedding):
    """[L,B,T,D]+[B,T,D] -> per-row stacks [B*T, 25, XF] bf16 (row-major,
    rows padded to the XF pitch)."""
    lo_flat = layer_outputs.reshape(L, B * T, D)
    emb_flat = embedding.reshape(B * T, D)
    loT = np.zeros((B * T, NJ, XF), dtype=bfnp)
    loT[:, 0, :D] = emb_flat.astype(bfnp)
    loT[:, 1:, :D] = lo_flat.transpose(1, 0, 2).astype(bfnp)
    return loT


def _prep_xt(loT_core, qwT):
    """Per-core d-major stream: [128, NBATCH, NCHUNK, CW] flattened, where
    xt[p, b, c, j] = X_b[j, c*128+p] for j<125, qw^T columns at 125:149."""
    out = np.zeros((128, NBATCH, NCHUNK, CW), dtype=bfnp)
    # batches 0..50 tile rows 0..254 contiguously
    x51 = loT_core[: 51 * R, :, :D].reshape(51, P, NCHUNK, 128)
    out[:, :51, :, 0:P] = x51.transpose(3, 0, 2, 1)
    # final overlap batch starts at row 251
    row0 = _batch_starts()[-1]
    xl = loT_core[row0 : row0 + R, :, :D].reshape(P, NCHUNK, 128)
    out[:, 51, :, 0:P] = xl.transpose(2, 1, 0)
    out[:, :, :, P:SCW] = qwT.reshape(128, NCHUNK, L)[:, None, :, :]
    return np.ascontiguousarray(out.reshape(128, NBATCH * NCHUNK * CW))


def _make_in_maps(layer_outputs, embedding, queries, key_norm_weight):
    loT = _prep_loT(layer_outputs, embedding)
    consts = _build_consts(queries, key_norm_weight)
    in_maps = []
    pad3 = np.zeros((3, XF), dtype=bfnp)
    for c in range(N_CORES):
        r0 = c * ROWS_PER_CORE
        core_loT = loT[r0 : r0 + ROWS_PER_CORE]
        in_maps.append({
            "loT": np.concatenate(
                [core_loT.reshape(ROWS_PER_CORE * NJ, XF), pad3], axis=0
            ),
            "xtT": _prep_xt(core_loT, consts["qwT"]),
            "mtbd": consts["mtbd"],
            "mbd": consts["mbd"],
            "mbdb": consts["mbdb"],
            "diagm": consts["diagm"],
            "maskneg": consts["maskneg"],
            "ident": consts["ident"],
        })
    return in_maps


def kernel(layer_outputs, embedding, queries, key_norm_weight):
    global _NC_CACHE
    layer_outputs = np.asarray(layer_outputs, dtype=np.float32)
    embedding = np.asarray(embedding, dtype=np.float32)
    queries = np.asarray(queries, dtype=np.float32)
    key_norm_weight = np.asarray(key_norm_weight, dtype=np.float32)

    in_maps = _make_in_maps(layer_outputs, embedding, queries, key_norm_weight)

    if _NC_CACHE is None:
        _NC_CACHE = build_kernel()
    nc = _NC_CACHE

    res = run_bass_kernel_spmd(nc, in_maps, core_ids=list(range(N_CORES)))

    full = np.empty((L, B * T, D), dtype=np.float32)
    for c in range(N_CORES):
        r0 = c * ROWS_PER_CORE
        outT = res.results[c]["outT"].reshape(ROWS_PER_CORE, L, XF)[:, :, :D]
        full[:, r0 : r0 + ROWS_PER_CORE, :] = (
            outT.astype(np.float32).transpose(1, 0, 2)
        )
    return full.reshape(L, B, T, D)
